# revision 17
# baseline (speedup 1.0000x reference)
"""Distributed Trainium2 Bass kernel for causal multi-head attention with RoPE.

Reference computation (B=2, S=2048, E=1024, H=16, D=64, fp32):
    q = rope((x @ Wq.T).heads); k = rope((x @ Wk.T).heads); v = (x @ Wv.T).heads
    out = softmax(mask(q k^T / sqrt(E))) v  -> concat heads -> @ Wo.T

Sharding (8 NeuronCores): data parallel over B (2 groups of 4 cores),
tensor parallel over heads within each group (4 heads per core).
Each core computes QKV for its 4 heads, flash-style causal attention,
normalized attention output transposed (d x s), AllGathers per-chunk
across its 4-rank group, then computes a 256-col slice of Wo.

Schedule notes (per-engine queues are in-order; Tile orders by priority):
  - QK projection for seq-block st is issued, then attention chunks in
    that range: exp (ACT) overlaps the PE matmuls of the next block.
  - The kb loop is software-pipelined 3 deep (PSUM pool bufs=3) so the
    PE computes scores kb+1..kb+3 while ACT runs exp(kb).
  - Diagonal score blocks are narrowed to the causal region; the
    remaining partial 128-col block is masked with one triangular
    constant via a DVE multiply.
  - Wo blocks are pushed to the end of the schedule with
    tile_wait_until so the PE never stalls mid-stream on an AllGather.
  - A tiny high-priority warm-up AllGather absorbs the collective
    stream entry barrier + ncfw cold start during phase 1.
  - The last 512 sq block is split into two 256 chunks so the
    tail-exposed final AllGather is half size.

Host-side prep (per-core input shards):
  - x fed transposed (E,S) in bf16.
  - Wq/Wk rows permuted per head to de-interleave RoPE pairs (even dims
    first, odd dims second) so RoPE becomes the rotate-half form.
  - cos/sin tables (bf16), the 32-row swap matrix and the [128,128]
    causal triangle mask are precomputed constants.
"""

import os
import sys

sys.path.insert(0, "/opt/trn_rl_repo")

import numpy as np
import ml_dtypes

import concourse.bass as bass
import concourse.bacc as bacc
import concourse.mybir as mybir
import concourse.tile as tile
from concourse import bass_utils

B, S, E, H, D = 2, 2048, 1024, 16, 64
NCORES = 8
TP = 4                 # tensor-parallel group size
HPC = H // TP          # heads per core = 4
DQ = HPC * D           # per-core projection width = 256
ATTN_SCALE = 1.0 / float(np.sqrt(E))

FP32 = mybir.dt.float32
BF16 = mybir.dt.bfloat16

SQT = 512              # sq block width (qk projection granularity)
SKB = 128              # sk block (partition dim of score tiles)
NSQT = S // SQT        # 4
NST16 = S // 128       # 16
NE = E // 128          # 8 contraction steps

# attention/AllGather chunks: (sq0, cw), in processing order. The tiny
# (0, 256) chunk runs LAST: its ~2us of attention means the final
# (tail-exposed) AllGather fires right after the previous one.
CHUNKS = [(256, 256), (512, 512), (1024, 512), (1536, 512), (0, 256)]
# chunks to run after each qk st block (keys for chunk ci must be ready)
CHUNK_PLAN = {0: [0], 1: [1], 2: [2], 3: [3, 4]}

REPLICA_GROUPS = [[0, 1, 2, 3], [4, 5, 6, 7]]

_CACHE = {}
LAST_RESULT = None


def build_nc():
    nc = bacc.Bacc(None, target_bir_lowering=False)

    xT = nc.declare_dram_parameter("xT", [E, S], BF16, isOutput=False)
    wqT = nc.declare_dram_parameter("wqT", [E, DQ], BF16, isOutput=False)
    wkT = nc.declare_dram_parameter("wkT", [E, DQ], BF16, isOutput=False)
    wvT = nc.declare_dram_parameter("wvT", [E, DQ], BF16, isOutput=False)
    woT = nc.declare_dram_parameter("woT", [E, DQ], BF16, isOutput=False)
    cosd = nc.declare_dram_parameter("cos", [128, S], BF16, isOutput=False)
    sind = nc.declare_dram_parameter("sin", [128, S], BF16, isOutput=False)
    swapd = nc.declare_dram_parameter("swapmat", [128, 128], BF16, isOutput=False)
    maskd = nc.declare_dram_parameter("trimask", [128, 128], BF16, isOutput=False)
    # transposed output (DQ, S); the host transposes back during unshard
    out_ext = nc.declare_dram_parameter("out", [DQ, S], FP32, isOutput=True)

    with tile.TileContext(nc) as tc:
        with (
            tc.tile_pool(name="dram", bufs=1, space="DRAM") as drampool,
            tc.tile_pool(name="const", bufs=1) as constpool,
            tc.tile_pool(name="psA", bufs=3, space="PSUM") as psApool,
            tc.tile_pool(name="psO", bufs=1, space="PSUM") as psOpool,
            tc.tile_pool(name="ptp", bufs=3) as ptpool,
            tc.tile_pool(name="rope", bufs=3) as rtpool,
            tc.tile_pool(name="fin", bufs=2) as finpool,
            tc.tile_pool(name="gtp", bufs=2) as gtpool,
            tc.tile_pool(name="osb", bufs=2) as osbpool,
        ):
            # ---- persistent SBUF tensors ----
            w_sb = {}
            for name in ("wq", "wk", "wv", "wo"):
                w_sb[name] = constpool.tile(
                    [128, NE * DQ], BF16, tag=f"w_{name}", name=f"w_{name}"
                )
            cos_sb = constpool.tile([128, S], BF16, tag="cos")
            sin_sb = constpool.tile([128, S], BF16, tag="sin")
            swap_sb = constpool.tile([128, 128], BF16, tag="swap")
            mask_sb = constpool.tile([128, 128], BF16, tag="mask")
            xt = [
                constpool.tile([128, S], BF16, tag=f"xT{j}", name=f"xT{j}")
                for j in range(NE)
            ]
            qt_sb = [
                constpool.tile([128, S], BF16, tag=f"qt{g}", name=f"qt{g}")
                for g in range(2)
            ]
            kt_sb = [
                constpool.tile([128, S], BF16, tag=f"kt{g}", name=f"kt{g}")
                for g in range(2)
            ]
            vaug = [
                constpool.tile([128, HPC * 65], BF16, tag=f"vaug{i}", name=f"vaug{i}")
                for i in range(NST16)
            ]
            # all 4 heads' attn^T concatenated along free dim: [:, h*S + s]
            attnT = constpool.tile([64, HPC * S], BF16, tag="attnT", name="attnT")

            def load_w(name, dram):
                # one DMA: w_sb[p, j*DQ + c] = dram[j*128 + p, c]
                nc.sync.dma_start(
                    out=w_sb[name][:].rearrange("p (j c) -> p j c", j=NE),
                    in_=dram.rearrange("(j p) c -> p j c", j=NE),
                )

            # ------- warm-up AllGather: first on the gpsimd queue -------
            warm_in = drampool.tile([64, 8], BF16, tag="warm_in", name="warm_in")
            warm_out = drampool.tile(
                [256, 8], BF16, tag="warm_out", name="warm_out",
            )
            with tc.high_priority():
                nc.gpsimd.collective_compute(
                    "AllGather",
                    mybir.AluOpType.bypass,
                    ins=[warm_in.opt()],
                    outs=[warm_out.opt()],
                    replica_groups=REPLICA_GROUPS,
                )

            # ---------------- input loads (SP queue) ----------------
            # first half of x feeds V tiles 0-7, qk st 0-1, attn chunks 0-1
            load_w("wv", wvT)
            for j in range(NE):
                nc.sync.dma_start(
                    out=xt[j][:, 0:S // 2], in_=xT[j * 128:(j + 1) * 128, 0:S // 2]
                )
            load_w("wq", wqT)
            load_w("wk", wkT)
            nc.sync.dma_start(out=swap_sb[:], in_=swapd[:])
            nc.sync.dma_start(out=cos_sb[:], in_=cosd[:])
            nc.sync.dma_start(out=sin_sb[:], in_=sind[:])
            nc.sync.dma_start(out=mask_sb[:], in_=maskd[:])
            for j in range(NE):
                nc.sync.dma_start(
                    out=xt[j][:, S // 2:S], in_=xT[j * 128:(j + 1) * 128, S // 2:S]
                )
            load_w("wo", woT)

            for i in range(NST16):
                nc.gpsimd.memset(vaug[i][:], 1.0)

            # ---------------- V projection (spread through the stream) ----
            def v_block(i):
                psv = psApool.tile([128, 2 * SQT], FP32, tag="psA", name=f"psv{i}")
                for j in range(NE):
                    nc.tensor.matmul(
                        psv[:, 0:DQ],
                        lhsT=xt[j][:, i * 128:(i + 1) * 128],
                        rhs=w_sb["wv"][:, j * DQ:(j + 1) * DQ],
                        start=(j == 0),
                        stop=(j == NE - 1),
                    )
                # one strided copy drops V into the 4 per-head 65-wide
                # slots, leaving column 64 of each slot at 1.0
                nc.vector.tensor_copy(
                    vaug[i][:, 0:HPC * 65].rearrange(
                        "p (h w) -> p h w", h=HPC
                    )[:, :, 0:64],
                    psv[:, 0:DQ].rearrange("p (h w) -> p h w", h=HPC),
                )

            # ---------- interleaved QK projection+RoPE / attention ----------
            ag_bufs = []  # (agout, cw, sq0) in chunk order

            def qk_block(g, st):
                sq = slice(st * SQT, (st + 1) * SQT)
                for wname, dst in (("wq", qt_sb), ("wk", kt_sb)):
                    ps = psApool.tile(
                        [128, 2 * SQT], FP32, tag="psA", name=f"ps_{wname}{g}_{st}"
                    )
                    for j in range(NE):
                        nc.tensor.matmul(
                            ps[:, 0:SQT],
                            lhsT=w_sb[wname][
                                :, j * DQ + g * 128: j * DQ + g * 128 + 128
                            ],
                            rhs=xt[j][:, sq],
                            start=(j == 0),
                            stop=(j == NE - 1),
                        )
                    raw = rtpool.tile([128, SQT], BF16, tag="raw")
                    nc.vector.tensor_copy(raw[:], ps[:, 0:SQT])
                    nc.tensor.matmul(
                        ps[:, SQT:2 * SQT], lhsT=swap_sb[:], rhs=raw[:],
                        start=True, stop=True,
                    )
                    t1 = rtpool.tile([128, SQT], FP32, tag="t1")
                    nc.vector.tensor_mul(t1[:], ps[:, SQT:2 * SQT], sin_sb[:, sq])
                    t2 = rtpool.tile([128, SQT], BF16, tag="t2")
                    nc.vector.tensor_mul(t2[:], raw[:], cos_sb[:, sq])
                    nc.vector.tensor_add(dst[g][:, sq], t1[:], t2[:])

            def attn_chunk(ci):
                sq0, cw = CHUNKS[ci]
                nblk = (sq0 + cw) // SKB
                for g in range(2):
                    pso = [
                        psOpool.tile([65, cw], FP32, tag=f"pso{p}",
                                     name=f"pso{p}_{g}_{ci}")
                        for p in range(2)
                    ]
                    pss_tiles = {}

                    def emit_scores(kb):
                        d = max(0, kb * SKB - sq0)
                        pss = psApool.tile([128, 2 * SQT], FP32, tag="psA",
                                           name=f"pss_{g}_{ci}_{kb}")
                        for p in range(2):
                            nc.tensor.matmul(
                                pss[:, p * SQT + d: p * SQT + cw],
                                lhsT=kt_sb[g][
                                    p * 64:(p + 1) * 64,
                                    kb * SKB:(kb + 1) * SKB,
                                ],
                                rhs=qt_sb[g][p * 64:(p + 1) * 64,
                                             sq0 + d: sq0 + cw],
                                start=True,
                                stop=True,
                            )
                        pss_tiles[kb] = (pss, d)

                    for kb in range(min(3, nblk)):
                        emit_scores(kb)
                    for kb in range(nblk):
                        pss, d = pss_tiles.pop(kb)
                        pt = ptpool.tile([128, 2 * SQT], BF16, tag="pt",
                                         name=f"pt_{g}_{ci}_{kb}")
                        # exp over both heads with a (2, cw-d) strided AP
                        pt3 = pt[:].rearrange("p (h c) -> p h c", h=2)[:, :, d:cw]
                        pss3 = pss[:].rearrange("p (h c) -> p h c", h=2)[:, :, d:cw]
                        nc.scalar.activation(
                            pt3, pss3,
                            mybir.ActivationFunctionType.Exp,
                            scale=ATTN_SCALE,
                        )
                        if kb * SKB >= sq0:
                            # partial diagonal 128-col block starts at d
                            for p in range(2):
                                sl = slice(p * SQT + d, p * SQT + d + SKB)
                                nc.vector.tensor_mul(
                                    pt[:, sl], pt[:, sl], mask_sb[:]
                                )
                        for p in range(2):
                            h = 2 * g + p
                            nc.tensor.matmul(
                                pso[p][:, d:cw],
                                lhsT=vaug[kb][:, h * 65:(h + 1) * 65],
                                rhs=pt[:, p * SQT + d: p * SQT + cw],
                                start=(kb == 0),
                                stop=(kb == nblk - 1),
                            )
                        if kb + 3 < nblk:
                            emit_scores(kb + 3)
                    # evacuate pso fast, then the reciprocal chain
                    un = []
                    lrow = []
                    for p in range(2):
                        lr = finpool.tile([1, cw], FP32, tag=f"lrow{p}",
                                          name=f"lrow{p}_{g}_{ci}")
                        nc.vector.tensor_copy(lr[:], pso[p][64:65, :])
                        u = finpool.tile([64, cw], BF16, tag=f"un{p}",
                                         name=f"un{p}_{g}_{ci}")
                        nc.vector.tensor_copy(u[:], pso[p][0:64, :])
                        un.append(u)
                        lrow.append(lr)
                    for p in range(2):
                        h = 2 * g + p
                        linv = finpool.tile([1, cw], FP32, tag=f"linv{p}")
                        nc.vector.reciprocal_approx_fast(linv[:], lrow[p][:])
                        lbc = finpool.tile([64, cw], FP32, tag=f"lbc{p}")
                        nc.gpsimd.partition_broadcast(lbc[:], linv[:])
                        nc.vector.tensor_mul(
                            attnT[:, h * S + sq0: h * S + sq0 + cw],
                            un[p][:], lbc[:],
                        )
                # ---- AllGather this chunk ----
                agin = drampool.tile(
                    [DQ, cw], BF16, tag=f"agin{ci}", name=f"agin{ci}"
                )
                agout = drampool.tile(
                    [E, cw], BF16, tag=f"agout{ci}", name=f"agout{ci}"
                )
                nc.gpsimd.dma_start(
                    out=agin.rearrange("(h p) c -> p h c", h=HPC),
                    in_=attnT[:].rearrange(
                        "p (h s) -> p h s", h=HPC
                    )[:, :, sq0:sq0 + cw],
                )
                nc.gpsimd.collective_compute(
                    "AllGather",
                    mybir.AluOpType.bypass,
                    ins=[agin.opt()],
                    outs=[agout.opt()],
                    replica_groups=REPLICA_GROUPS,
                )
                ag_bufs.append((agout, cw, sq0))

            # all V blocks run during/right after the input DMA window,
            # before the attention stream starts
            for i in range(NST16):
                v_block(i)
            for st in range(NSQT):
                for g in range(2):
                    qk_block(g, st)
                for ci in CHUNK_PLAN[st]:
                    attn_chunk(ci)

            # ---------------- Wo tail (transposed output) ----------------
            # tile_wait_until pushes these past all attention work in the
            # scheduler's model so the PE stream never blocks on an AG.
            # psw_T[dq, s] = Wo_slice^T gathered-attn: N=cw matmuls, and the
            # [DQ, S] output layout makes the store DMA contiguous.
            for wi, (agout, cw, sq0) in enumerate(ag_bufs):
                with tc.tile_wait_until(0.5 + 0.02 * wi):
                    gt = gtpool.tile([128, NE * cw], BF16, tag="gtall",
                                     name=f"gt{sq0}")
                    for half in range(2):
                        jsl = slice(half * (NE // 2), (half + 1) * (NE // 2))
                        nc.sync.dma_start(
                            out=gt[:].rearrange(
                                "p (j c) -> p j c", j=NE)[:, jsl],
                            in_=agout.rearrange(
                                "(j p) c -> p j c", j=NE)[:, jsl],
                        )
                    osb = osbpool.tile([128, 2 * SQT], FP32, tag="osb",
                                       name=f"osb{sq0}")
                    for dqh in range(2):
                        psw = psApool.tile([128, 2 * SQT], FP32, tag="psA",
                                           name=f"psw{sq0}_{dqh}")
                        for j in range(NE):
                            nc.tensor.matmul(
                                psw[:, 0:cw],
                                lhsT=w_sb["wo"][
                                    :, j * DQ + dqh * 128:
                                    j * DQ + (dqh + 1) * 128],
                                rhs=gt[:, j * cw:(j + 1) * cw],
                                start=(j == 0),
                                stop=(j == NE - 1),
                            )
                        nc.vector.tensor_copy(
                            osb[:, dqh * cw:dqh * cw + cw], psw[:, 0:cw]
                        )
                        nc.scalar.dma_start(
                            out=out_ext[dqh * 128:(dqh + 1) * 128,
                                        sq0:sq0 + cw],
                            in_=osb[:, dqh * cw:dqh * cw + cw],
                        )

    nc.finalize()
    return nc


def _host_tables():
    inv = 1.0 / (10000.0 ** (np.arange(0, D, 2, dtype=np.float64) / D))  # (32,)
    ang = np.arange(S, dtype=np.float64)[None, :] * inv[:, None]          # (32,S)
    cos32 = np.cos(ang)
    sin32 = np.sin(ang)
    cos = np.tile(cos32, (4, 1)).astype(np.float32)                       # (128,S)
    sin = np.concatenate([-sin32, sin32, -sin32, sin32], axis=0).astype(np.float32)
    swap = np.zeros((128, 128), np.float32)
    for k in range(128):
        blk = (k // 64) * 64
        swap[k, blk + ((k - blk) + 32) % 64] = 1.0
    # causal triangle for the partial diagonal block: keep col >= row
    tri = (np.arange(128)[None, :] >= np.arange(128)[:, None]).astype(np.float32)
    return cos, sin, swap, tri


def kernel(x, W_q, W_k, W_v, W_o):
    global LAST_RESULT
    if "nc" not in _CACHE:
        _CACHE["nc"] = build_nc()
    nc = _CACHE["nc"]

    bf = ml_dtypes.bfloat16
    perm = np.concatenate([np.arange(0, D, 2), np.arange(1, D, 2)])
    rowperm = (np.arange(H)[:, None] * D + perm[None, :]).reshape(-1)
    Wq_p = W_q[rowperm]
    Wk_p = W_k[rowperm]
    cos, sin, swap, tri = _host_tables()

    in_maps = []
    for c in range(NCORES):
        b, tp = c // TP, c % TP
        sl = slice(tp * DQ, (tp + 1) * DQ)
        in_maps.append({
            "xT": np.ascontiguousarray(x[b].T).astype(bf),
            "wqT": np.ascontiguousarray(Wq_p[sl].T).astype(bf),
            "wkT": np.ascontiguousarray(Wk_p[sl].T).astype(bf),
            "wvT": np.ascontiguousarray(W_v[sl].T).astype(bf),
            "woT": np.ascontiguousarray(W_o[sl].T).astype(bf),
            "cos": cos.astype(bf),
            "sin": sin.astype(bf),
            "swapmat": swap.astype(bf),
            "trimask": tri.astype(bf),
        })

    res = bass_utils.run_bass_kernel_spmd(
        nc, in_maps, core_ids=list(range(NCORES)),
        tmpdir=os.environ.get("BASS_TMPDIR") or None,
    )
    LAST_RESULT = res
    out = np.empty((B, S, E), np.float32)
    for c in range(NCORES):
        b, tp = c // TP, c % TP
        out[b][:, tp * DQ:(tp + 1) * DQ] = np.asarray(
            res.results[c]["out"], dtype=np.float32
        ).T
    return out
